# revision 3
# baseline (speedup 1.0000x reference)
"""AttentiveManifoldMixer Trainium2 kernel (8-core data parallel over batch).

Baseline structure (sequential halves, on-device window DMAs, two HWDGE
queues, per-half PSUM banks) with a 16-chunk pair cover:

Math: with W3[c,i,j] = conv_w[c*64+i, j], Bmat = conv_b.reshape(C, C),
  s[b]       = sigmoid(fc2 @ relu(fc1 @ mean_hw(x[b])))
  out[b,c,p] = sum_{i,j} W3[c,i,j] s[b,j] x[b,i,p] x[b,j,p]
               + sum_i Bmat[c,i] x[b,i,p]

Off-diagonal pairs ride 16 feature chunks (4 A-windows x 4 B-windows of
[x;x]).  Chunk k = 4l + t maps lane q = 64*qhi + qlo to
  i = (qlo + a_t) % 64,  a in {0, 56, 48, 40}   (A_0 is X2 itself)
  j = (qlo + c_l + qhi) % 64,  c in {1, 3, 5, 7}
so d = j-i covers 1..32 exactly once (d=32 lanes doubled, mult=2).  The
diagonal (i==j) and the conv_b residual ride one extra matmul per half
whose rhs is U2 = [x; x^2] (built by an ACT copy that doubles as the SE
channel-sum reduction, plus an ACT Square) and lhsT = [Bmat.T; diag*s].
"""
import sys

sys.path.insert(0, "/opt/trn_rl_repo")

import numpy as np
import ml_dtypes

B, C, H, W = 8, 64, 64, 64
P = H * W
MID = C // 4
NCHUNK = 16                # chunk k = 4l + t
A_ROWS = [0, 56, 48, 40]   # i = (qlo + a_t) % 64; a_0 -> X2 directly
B_COLS = [1, 3, 5, 7]      # j = (qlo + c_l + qhi) % 64
NSUB = 512
NSPLIT = 2
HALF = P // NSPLIT
NBANK = HALF // (2 * NSUB)
N_CORES = 8
NT = 8                     # V slots: 0-3 B-tiles, 4-6 A-tiles, 7 U2

_CACHE = {}


def _tl(k):
    return k % 4, k // 4


def _lane_maps():
    i_idx = np.zeros((NCHUNK, 128), np.int64)
    j_idx = np.zeros((NCHUNK, 128), np.int64)
    for k in range(NCHUNK):
        t, l = _tl(k)
        for q in range(128):
            qhi, qlo = divmod(q, 64)
            i_idx[k, q] = (qlo + A_ROWS[t]) % C
            j_idx[k, q] = (qlo + B_COLS[l] + qhi) % C
    lo = np.minimum(i_idx, j_idx)
    hi = np.maximum(i_idx, j_idx)
    key = lo * C + hi
    _, inv, counts = np.unique(key, return_inverse=True, return_counts=True)
    mult = counts[inv].reshape(key.shape).astype(np.float32)
    return i_idx, j_idx, mult


def _host_weights(conv_w, fc1_w, fc2_w):
    w3 = conv_w.reshape(C, C, C)  # [c, i, j]
    i_idx, j_idx, mult = _lane_maps()
    a1 = np.transpose(w3[:, i_idx, j_idx], (2, 1, 0)) / mult.T[:, :, None]
    a2 = np.transpose(w3[:, j_idx, i_idx], (2, 1, 0)) / mult.T[:, :, None]
    diagT = np.ascontiguousarray(
        np.transpose(w3[:, np.arange(C), np.arange(C)], (1, 0)), np.float32)
    fc1t = (fc1_w.T / float(P)).copy()
    fc2t = fc2_w.T.copy()
    return (np.ascontiguousarray(a1, ml_dtypes.bfloat16),
            np.ascontiguousarray(a2, ml_dtypes.bfloat16), diagT, fc1t, fc2t)


def _host_perm():
    """[64, 8, 128]: col l<4 -> s[(qlo + c_l + qhi) % 64]; col 4+t ->
    s[(qlo + a_t) % 64]."""
    pm = np.zeros((C, 8, 128), np.float32)
    q = np.arange(128)
    qhi, qlo = q // 64, q % 64
    for n in range(8):
        idx = ((qlo + B_COLS[n] + qhi) % C if n < 4
               else (qlo + A_ROWS[n - 4]) % C)
        pm[idx, n, q] = 1.0
    return pm


def _host_idb(conv_b):
    bt = np.asarray(conv_b, np.float32).reshape(C, C).T
    return np.ascontiguousarray(bt, ml_dtypes.bfloat16)


def _build_program(niter=None):
    import contextlib

    import concourse.bacc as bacc
    import concourse.bass as bass
    from concourse import mybir
    from concourse.tile import TileContext

    nc = bacc.Bacc("TRN2", target_bir_lowering=False, debug=False)
    dt = mybir.dt
    AF = mybir.ActivationFunctionType

    x_d = nc.dram_tensor("x", [128, P], dt.bfloat16, kind="ExternalInput")
    aw_d = nc.dram_tensor("aw", [128, 2, NCHUNK, C], dt.bfloat16,
                          kind="ExternalInput")
    f1_d = nc.dram_tensor("fc1t", [C, MID], dt.float32, kind="ExternalInput")
    f2_d = nc.dram_tensor("fc2t", [MID, C], dt.float32, kind="ExternalInput")
    id_d = nc.dram_tensor("ident", [C, C], dt.bfloat16, kind="ExternalInput")
    dg_d = nc.dram_tensor("diagT", [C, C], dt.float32, kind="ExternalInput")
    pm_d = nc.dram_tensor("perm", [C, 8, 128], dt.float32, kind="ExternalInput")
    out_d = nc.dram_tensor("out", [C, P], dt.bfloat16, kind="ExternalOutput")

    with TileContext(nc) as tc:
        with tc.tile_pool(name="single", bufs=1) as single, \
             tc.tile_pool(name="feat", bufs=16) as featp, \
             tc.tile_pool(name="outs", bufs=6) as outsp, \
             tc.tile_pool(name="psum", bufs=8, space="PSUM") as psum, \
             (tc.For_i(0, niter, 1,
                       hint_engines=(mybir.EngineType.PE,
                                     mybir.EngineType.DVE,
                                     mybir.EngineType.SP,
                                     mybir.EngineType.Pool,
                                     mybir.EngineType.Activation))
              if niter else contextlib.nullcontext()):

            hsls = [slice(i * HALF, (i + 1) * HALF) for i in range(NSPLIT)]

            # early sigmoid-table trigger
            dum = single.tile([1, 1], dt.float32)
            nc.gpsimd.memset(dum, 0.0)
            nc.scalar.activation(dum, dum, AF.Sigmoid)

            aws = single.tile([128, 2, NCHUNK, C], dt.bfloat16)
            a1s = aws[:, 0]
            a2s = aws[:, 1]
            f1s = single.tile([C, MID], dt.float32)
            nc.scalar.dma_start(out=f1s, in_=f1_d.ap())
            f2s = single.tile([MID, C], dt.float32)
            nc.scalar.dma_start(out=f2s, in_=f2_d.ap())
            ids = single.tile([128, C], dt.bfloat16)
            nc.scalar.dma_start(out=ids[0:C], in_=id_d.ap())
            dgs = single.tile([C, C], dt.float32)
            nc.scalar.dma_start(out=dgs, in_=dg_d.ap())
            pms = single.tile([C, 8, 128], dt.float32)
            nc.scalar.dma_start(out=pms, in_=pm_d.ap())
            nc.scalar.dma_start(out=aws, in_=aw_d.ap())

            # ---- prestage: [x;x] halves + window tiles via SBUF->SBUF
            # partition-window DMAs (order matches chunk consumption:
            # B0, A1, A2, A3, B1, B2, B3); U2 via ACT (copy emits SE sums).
            sums_h = [single.tile([C, 1], dt.float32, name=f"sums{h}")
                      for h in range(NSPLIT)]
            X2 = single.tile([128, P], dt.bfloat16)
            V = single.tile([128, NT, P], dt.bfloat16)
            VP = NT * P

            def vdst(qhi, slot, hsl):
                return bass.AP(tensor=V.tensor,
                               offset=V.offset + 64 * qhi * VP + slot * P
                               + hsl.start,
                               ap=[[VP, 64], [1, HALF]])

            def xwin(row0, hsl):
                return bass.AP(tensor=X2.tensor,
                               offset=X2.offset + row0 * P + hsl.start,
                               ap=[[P, 64], [1, HALF]])

            for h, hsl in enumerate(hsls):
                nc.sync.dma_start(out=X2[:, hsl], in_=x_d.ap()[:, hsl])
                # U2 lower (= x, emits channel sums); upper (= x^2)
                nc.scalar.activation(
                    bass.AP(tensor=V.tensor,
                            offset=V.offset + 7 * P + hsl.start,
                            ap=[[VP, C], [1, HALF]]),
                    X2[0:C, hsl], AF.Copy, accum_out=sums_h[h])
                nc.scalar.activation(
                    bass.AP(tensor=V.tensor,
                            offset=V.offset + 64 * VP + 7 * P + hsl.start,
                            ap=[[VP, C], [1, HALF]]),
                    X2[0:C, hsl], AF.Square)
                # windows in consumption order (l-minor sweep: B first)
                order = [(l, B_COLS[l], B_COLS[l] + 1) for l in (0, 1, 2, 3)]
                order += [(3 + t, A_ROWS[t], A_ROWS[t]) for t in (1, 2, 3)]
                for slot, wlo, whi in order:
                    nc.sync.dma_start(out=vdst(0, slot, hsl),
                                      in_=xwin(wlo, hsl))
                    nc.sync.dma_start(out=vdst(1, slot, hsl),
                                      in_=xwin(whi, hsl))

            # ---- SE path ----
            ps1 = psum.tile([MID, 1], dt.float32, tag="acc")
            for h in range(NSPLIT):
                nc.tensor.matmul(ps1, f1s, sums_h[h], start=(h == 0),
                                 stop=(h == NSPLIT - 1))
            y1 = single.tile([MID, 1], dt.float32)
            nc.scalar.activation(y1, ps1, AF.Relu)
            ps2 = psum.tile([C, 1], dt.float32, tag="acc")
            nc.tensor.matmul(ps2, f2s, y1, start=True, stop=True)
            svec = single.tile([C, 1], dt.float32)
            nc.scalar.activation(svec, ps2, AF.Sigmoid)

            sexp_ps = psum.tile([128, 8], dt.float32, tag="acc")
            for n in range(8):
                nc.tensor.matmul(sexp_ps[:, n:n + 1], pms[:, n, :], svec,
                                 start=True, stop=True)
            s12 = single.tile([128, 8], dt.float32)
            nc.scalar.copy(s12, sexp_ps)
            s1b = s12[:, 0:4]
            s2b = s12[:, 4:8]

            # wdiag into ids rows 64:128
            nc.scalar.mul(ids[64:128], dgs, svec)

            # ---- fold: wc[:,k,:] = a1*s_j(l) + a2*s_i(t), k = 4l + t ----
            wc = single.tile([128, NCHUNK, C], dt.bfloat16)
            t1 = single.tile([128, NCHUNK, C], dt.float32)
            t2 = single.tile([128, NCHUNK, C], dt.float32)
            for t in range(4):
                nc.scalar.mul(t2[:, t::4, :], a2s[:, t::4, :], s2b[:, t:t + 1])
            wcf = wc.rearrange("p a b -> p (a b)")
            t1f = t1.rearrange("p a b -> p (a b)")
            t2f = t2.rearrange("p a b -> p (a b)")
            for l in range(4):
                nc.scalar.mul(t1[:, 4 * l:4 * l + 4, :],
                              a1s[:, 4 * l:4 * l + 4, :], s1b[:, l:l + 1])
                nc.gpsimd.tensor_add(wcf[:, 4 * l * C:4 * (l + 1) * C],
                                     t1f[:, 4 * l * C:4 * (l + 1) * C],
                                     t2f[:, 4 * l * C:4 * (l + 1) * C])

            # ---- main sweep: per half, 16 feature TTs feed the GEMM ----
            NSH = HALF // NSUB
            for h, hsl in enumerate(hsls):
                banks = [psum.tile([C, NSUB], dt.float32, tag="acc",
                                   name=f"bank{h}_{j}") for j in range(NSH)]
                order16 = [0, 4, 8, 12, 1, 5, 9, 13, 2, 6, 10, 14,
                           3, 7, 11, 15]
                for pos, k in enumerate(order16):
                    t, l = _tl(k)
                    f = featp.tile([128, HALF], dt.bfloat16, tag="f")
                    ina = (X2[:, hsl] if t == 0 else
                           bass.AP(tensor=V.tensor,
                                   offset=V.offset + (3 + t) * P + hsl.start,
                                   ap=[[VP, 128], [1, HALF]]))
                    nc.vector.tensor_mul(
                        f, ina,
                        bass.AP(tensor=V.tensor,
                                offset=V.offset + l * P + hsl.start,
                                ap=[[VP, 128], [1, HALF]]))
                    for j in range(NSH):
                        nc.tensor.matmul(banks[j], wc[:, k, :],
                                         f[:, j * NSUB:(j + 1) * NSUB],
                                         start=(pos == 0),
                                         stop=(pos == NCHUNK - 1))
                    if pos == 8:
                        # U2 matmul mid-stream: += [Bmat.T; diag*s].T @ [x;x^2]
                        for j in range(NSH):
                            col = h * HALF + j * NSUB
                            nc.tensor.matmul(
                                banks[j], ids,
                                bass.AP(tensor=V.tensor,
                                        offset=V.offset + 7 * P + col,
                                        ap=[[VP, 128], [1, NSUB]]),
                                start=False, stop=False)
                pairs = ([(0, 2), (2, 2)] if h == 0 else
                         [(0, 2), (2, 1), (3, 1)])
                for j0, nj in pairs:
                    col = h * HALF + j0 * NSUB
                    ot = outsp.tile([C, nj * NSUB], dt.bfloat16, tag="o")
                    for jj in range(nj):
                        nc.scalar.copy(ot[:, jj * NSUB:(jj + 1) * NSUB],
                                       banks[j0 + jj])
                    nc.scalar.dma_start(
                        out=out_d.ap()[:, col:col + nj * NSUB], in_=ot)

    nc.compile()
    return nc


def _get_program(niter=None):
    key = ("nc", niter)
    if key not in _CACHE:
        _CACHE[key] = _build_program(niter)
    return _CACHE[key]


def _host_inputs(x, fc1_w, fc2_w, conv_w, conv_b):
    a1, a2, diagT, fc1t, fc2t = _host_weights(
        np.asarray(conv_w, np.float32), np.asarray(fc1_w, np.float32),
        np.asarray(fc2_w, np.float32))
    aw = np.ascontiguousarray(np.stack([a1, a2], axis=1))
    ident = _host_idb(conv_b)
    perm = _host_perm()
    in_maps = []
    for b in range(N_CORES):
        xb = np.asarray(x, np.float32)[b].reshape(C, P).astype(ml_dtypes.bfloat16)
        in_maps.append({
            "x": np.ascontiguousarray(np.concatenate([xb, xb], 0)),
            "aw": aw, "fc1t": fc1t, "fc2t": fc2t, "ident": ident,
            "diagT": diagT, "perm": perm,
        })
    return in_maps


def kernel(x, fc1_w, fc2_w, conv_w, conv_b):
    from concourse.bass_utils import run_bass_kernel_spmd

    in_maps = _host_inputs(x, fc1_w, fc2_w, conv_w, conv_b)
    nc = _get_program()
    res = run_bass_kernel_spmd(nc, in_maps, core_ids=list(range(N_CORES)))
    out = np.stack([res.results[b]["out"].reshape(C, H, W)
                    for b in range(N_CORES)], axis=0)
    return out.astype(np.float32)
